# revision 33
# baseline (speedup 1.0000x reference)
"""Trainium2 Bass kernel for Luong local-p sparse attention (v5).

Math (per batch n, full shapes N=64, L=258, H=1024, Q=256):
    score = (h_t @ W_a) @ enc^T           masked to window [p_t-16, p_t+16]
    align = softmax(score) * gauss(p_t)
    out   = tanh([align @ enc, h_t] @ W_c^T)

Only a 32-wide slice of enc can survive the mask for non-integer p_t, so the
kernel gathers 32-wide windows host-side (W=32 -> 4 windows pack exactly into
128 PE partitions).  Score path (u = W_a-transform of windows, score, softmax)
runs in fp32r for softmax accuracy; the heavy W_c matmuls (dec @ W_c2T, window
@ W_c1T, align @ v) run in bf16 (same PE rate, half the DMA/SBUF).

Layout: batch n's score/softmax lives on partitions [32*(n%4), 32*(n%4)+32) so
the align weights and the W_c1-transformed windows (vst tiles, 4 windows
stacked per 128 partitions) feed the context matmul directly as quadrant tiles
— no partition-shuffling DMAs anywhere.

Schedule ("dec_group-first"): the dec @ W_c2T matmuls for batches 0-3 need
only dec + W_c2T (3 MB), so after a short PE warm-up they run PE-bound from
~14us while enc/W_aT/W_c1T stream in behind them; their partials are
evacuated to SBUF (bf16) and re-injected at context time with an identity
matmul into the same PSUM accumulation.  Then u -> scores 0-3 interleaved
with v -> contexts 0-3, and batches 4-7 run the standard softly-pipelined
score/dec_group/ctx schedule.  Data parallel over batch: 8 per core.
"""

import numpy as np
import ml_dtypes

import concourse.bass as bass
import concourse.bacc as bacc
import concourse.mybir as mybir
import concourse.tile as tile
from concourse.bass_utils import run_bass_kernel_spmd

# Problem constants (hardcoded per harness contract).
N, L, H, Q = 64, 258, 1024, 256
WINDOW = 16.0
DEV_POW = 128.0
NCORES = 8
B = N // NCORES  # batches per core
W = 32           # window width (max live positions for non-integer p_t)
HC = H // 128    # h-chunks of 128 (PE contraction tiles)
F32 = mybir.dt.float32
F32R = mybir.dt.float32r
BF16 = mybir.dt.bfloat16
F16 = mybir.dt.float16
AF = mybir.ActivationFunctionType

# exp is computed as t = exp(s/4 + bias); bias = LOG_ALPHA keeps the
# column-sum T = sum_j t below fp32 max.  alpha cancels in w = t/T.
LOG_ALPHA = -4.8520302  # -7*ln(2)
MASK_BIAS = -10000.0    # exp(<= -2500) == 0 in fp32
N_WARM = 30             # PE warm-up matmuls spanning the initial DMA wait
NB_EARLY = 4            # batches whose dec_group runs ahead of u/score/v


def build_nc() -> bass.Bass:
    nc = bacc.Bacc()
    dec_bT = nc.declare_dram_parameter("dec_bT", [H, B * Q], BF16, isOutput=False)
    enc_wTb = nc.declare_dram_parameter("enc_wTb", [H, B * W], BF16, isOutput=False)
    W_aT = nc.declare_dram_parameter("W_aT", [H, H], BF16, isOutput=False)
    W_c1T = nc.declare_dram_parameter("W_c1T", [H, H], BF16, isOutput=False)
    W_c2T = nc.declare_dram_parameter("W_c2T", [H, H], BF16, isOutput=False)
    biasT = nc.declare_dram_parameter("biasT", [W, B], F32, isOutput=False)
    gPackT = nc.declare_dram_parameter("gPackT", [128, 2], F32, isOutput=False)
    onesD = nc.declare_dram_parameter("onesD", [W, W], F32R, isOutput=False)
    identD = nc.declare_dram_parameter("identD", [128, 128], BF16, isOutput=False)
    out = nc.declare_dram_parameter("out", [B * Q, H], F32, isOutput=True)

    with tile.TileContext(nc) as tc:
        with (
            tc.tile_pool(name="const", bufs=1) as cpool,
            tc.tile_pool(name="decb", bufs=4) as decb_pool,
            tc.tile_pool(name="dgp", bufs=NB_EARLY) as dgp_pool,
            tc.tile_pool(name="sm", bufs=2) as sm_pool,
            tc.tile_pool(name="outp", bufs=2) as out_pool,
            tc.tile_pool(name="psS", bufs=2, space="PSUM") as psS,
            tc.tile_pool(name="psB", bufs=6, space="PSUM") as psB,
        ):
            # ---------------- resident tensors ----------------
            encb_sb = cpool.tile([128, HC, B * W], BF16)
            WaT_sb = cpool.tile([128, HC, H], BF16)
            Wc1_sb = cpool.tile([128, HC, H], BF16)
            Wc2_sb = cpool.tile([128, HC, H], BF16)
            uT_sb = cpool.tile([128, HC, B * W], BF16)
            vst_sb = cpool.tile([128, 2, H], BF16)   # [4 windows, group, h']
            tb_sb = cpool.tile([128, 2, Q], BF16)    # align weights, stacked
            bias_sb = cpool.tile([W, B], F32)
            gpack_sb = cpool.tile([128, 2], F32)
            ones_sb = cpool.tile([W, W], F32R)
            id_sb = cpool.tile([128, 128], BF16)
            warm_sb = cpool.tile([128, 640], BF16)

            WaT_r = W_aT[:, :].rearrange("(c p) m -> p c m", p=128)
            Wc1_r = W_c1T[:, :].rearrange("(c p) m -> p c m", p=128)
            Wc2_r = W_c2T[:, :].rearrange("(c p) m -> p c m", p=128)
            decb_r = dec_bT[:, :].rearrange("(c p) (n q) -> p c n q", p=128, q=Q)
            encb_r = enc_wTb[:, :].rearrange("(c p) m -> p c m", p=128)

            decb_tiles = {}

            def load_decb(n):
                dt_ = decb_pool.tile([128, HC, Q], BF16, tag="decb", name=f"decb{n}")
                nc.sync.dma_start(out=dt_[:, 0:5, :], in_=decb_r[:, 0:5, n, :])
                nc.scalar.dma_start(out=dt_[:, 5:8, :], in_=decb_r[:, 5:8, n, :])
                decb_tiles[n] = dt_

            # ---------------- DMA schedule (issue order = priority) --------
            # tiny constants first (scalar ring)
            nc.scalar.dma_start(out=bias_sb, in_=biasT[:, :])
            nc.scalar.dma_start(out=gpack_sb, in_=gPackT[:, :])
            nc.scalar.dma_start(out=ones_sb, in_=onesD[:, :])
            nc.scalar.dma_start(out=id_sb, in_=identD[:, :])

            # dec_group-first deps: decb0, W_c2T, decb1-3 (all bf16)
            load_decb(0)
            nc.sync.dma_start(out=Wc2_sb[:, 0:3, :], in_=Wc2_r[:, 0:3, :])
            nc.scalar.dma_start(out=Wc2_sb[:, 3:5, :], in_=Wc2_r[:, 3:5, :])
            nc.sync.dma_start(out=Wc2_sb[:, 5:8, :], in_=Wc2_r[:, 5:8, :])
            for n in range(1, NB_EARLY):
                load_decb(n)

            # u deps: bf16 windows + W_aT (kc-major, both queues)
            nc.sync.dma_start(out=encb_sb[:, 0:5, :], in_=encb_r[:, 0:5, :])
            nc.scalar.dma_start(out=encb_sb[:, 5:8, :], in_=encb_r[:, 5:8, :])
            nc.sync.dma_start(out=WaT_sb[:, 0:3, :], in_=WaT_r[:, 0:3, :])
            nc.scalar.dma_start(out=WaT_sb[:, 3:5, :], in_=WaT_r[:, 3:5, :])
            nc.sync.dma_start(out=WaT_sb[:, 5:8, :], in_=WaT_r[:, 5:8, :])

            # v deps: W_c1T
            nc.scalar.dma_start(out=Wc1_sb[:, 0:4, :], in_=Wc1_r[:, 0:4, :])
            nc.sync.dma_start(out=Wc1_sb[:, 4:8, :], in_=Wc1_r[:, 4:8, :])

            # ---------------- PE warm-up ----------------
            # Long back-to-back matmuls cycling all 6 big PSUM slots (deep
            # pipelining hides slot-reuse semaphores); sustained PE busy trips
            # the HAM clock gate to 8/8 before real work starts.
            nc.vector.memset(warm_sb[:, :], 1.0)
            for i in range(N_WARM):
                pw = psB.tile([128, 512], F32, tag="B", name=f"warm{i}")
                nc.tensor.matmul(
                    pw, lhsT=warm_sb[:, 0:128], rhs=warm_sb[:, 128:640],
                    start=True, stop=True,
                )

            # ---------------- helpers ----------------
            def dg_mms(n, qt, ht, po, stop_last=False):
                db = decb_tiles[n]
                for hc in range(HC):
                    nc.tensor.matmul(
                        po,
                        lhsT=db[:, hc, qt * 128:(qt + 1) * 128],
                        rhs=Wc2_sb[:, hc, ht * 512:(ht + 1) * 512],
                        start=(hc == 0),
                        stop=(stop_last and hc == HC - 1),
                    )

            t_tiles = {}

            def score_part(n):
                dec_sb = decb_tiles[n]
                ps = psS.tile([W, Q], F32, tag="S", name=f"ps{n}")
                for hc in range(HC):
                    nc.tensor.matmul(
                        ps,
                        lhsT=uT_sb[:, hc, n * W:(n + 1) * W],
                        rhs=dec_sb[:, hc, :],
                        start=(hc == 0),
                        stop=(hc == HC - 1),
                    )
                t = sm_pool.tile([W, Q], F32R, tag="t", name=f"t{n}")
                nc.scalar.activation(
                    out=t, in_=ps, func=AF.Exp,
                    bias=bias_sb[:, n:n + 1], scale=0.25,
                )
                t_tiles[n] = t

            def smx_a(n):
                # first renorm pass: T = colsum t; t = (t/T)^4 (two squarings)
                t = t_tiles[n]
                pT = psS.tile([W, Q], F32, tag="S", name=f"pT{n}")
                nc.tensor.matmul(pT, lhsT=ones_sb, rhs=t, start=True, stop=True)
                rT = sm_pool.tile([W, Q], F32, tag="r", name=f"rT{n}")
                nc.vector.reciprocal_approx_fast(out=rT, in_=pT)
                nc.vector.tensor_mul(t, t, rT)
                nc.vector.tensor_mul(t, t, t)
                nc.vector.tensor_mul(t, t, t)

            def smx_b(n):
                # second renorm pass -> bf16 align weights; a tiny SWDGE DMA
                # shifts them into the stacked [4 windows x 128] ctx layout
                o = (n % 4) * W
                g = n // 4
                t = t_tiles[n]
                pZ = psS.tile([W, Q], F32, tag="S", name=f"pZ{n}")
                nc.tensor.matmul(pZ, lhsT=ones_sb, rhs=t, start=True, stop=True)
                rZ = sm_pool.tile([W, Q], F32, tag="r", name=f"rZ{n}")
                nc.vector.reciprocal_approx_fast(out=rZ, in_=pZ)
                tbt = sm_pool.tile([W, Q], BF16, tag="tbt", name=f"tbt{n}")
                nc.vector.tensor_mul(tbt, t, rZ)
                nc.gpsimd.dma_start(out=tb_sb[o:o + W, g, :], in_=tbt)
                del t_tiles[n]

            def v_group(g, nt):
                pv = psB.tile([128, 512], F32, tag="B", name=f"pv{g}_{nt}")
                for kc in range(HC):
                    nc.tensor.matmul(
                        pv,
                        lhsT=encb_sb[:, kc, g * 128:(g + 1) * 128],
                        rhs=Wc1_sb[:, kc, nt * 512:(nt + 1) * 512],
                        start=(kc == 0),
                        stop=(kc == HC - 1),
                    )
                # fold the gaussian; vst stays resident as the ctx rhs
                nc.vector.tensor_scalar_mul(
                    vst_sb[:, g, nt * 512:(nt + 1) * 512], pv, gpack_sb[:, g:g + 1]
                )

            def ctx_mm(n, qt, ht, po, start):
                o = (n % 4) * W
                g = n // 4
                nc.tensor.matmul(
                    po,
                    lhsT=tb_sb[o:o + W, g, qt * 128:(qt + 1) * 128],
                    rhs=vst_sb[o:o + W, g, ht * 512:(ht + 1) * 512],
                    start=start,
                    stop=not start,
                    tile_position=(o, 0),
                )

            # ---------------- phase 1: dec_group for batches 0-3 ----------
            # Partials evacuated to SBUF bf16; re-injected at context time.
            dgp_tiles = {}
            for n in range(NB_EARLY):
                dgp = dgp_pool.tile([128, 2, 2, 512], BF16, tag="dgp", name=f"dgp{n}")
                dgp_tiles[n] = dgp
                for qt, ht in ((0, 0), (1, 0), (0, 1), (1, 1)):
                    po = psB.tile([128, 512], F32, tag="B", name=f"pod{n}_{qt}_{ht}")
                    dg_mms(n, qt, ht, po, stop_last=True)
                    nc.scalar.copy(out=dgp[:, qt, ht, :], in_=po)

            # ---------------- phase 2: u  (uT[h, (n,j)], kc-major waves) ---
            for wave in range(2):
                pu = {}
                for kc in range(HC):
                    for ho in range(4):
                        hco = wave * 4 + ho
                        if kc == 0:
                            pu[hco] = psB.tile(
                                [128, B * W], F32, tag="B", name=f"pu{hco}"
                            )
                        nc.tensor.matmul(
                            pu[hco],
                            lhsT=WaT_sb[:, kc, hco * 128:(hco + 1) * 128],
                            rhs=encb_sb[:, kc, :],
                            start=(kc == 0),
                            stop=(kc == HC - 1),
                        )
                for ho in range(4):
                    hco = wave * 4 + ho
                    if ho % 2 == 0:
                        nc.scalar.copy(out=uT_sb[:, hco, :], in_=pu[hco])
                    else:
                        nc.vector.tensor_copy(out=uT_sb[:, hco, :], in_=pu[hco])

            # ------- phase 3: scores+softmax 0-3 interleaved with v ----
            v_group(0, 0)
            score_part(0)
            smx_a(0)
            score_part(1)
            load_decb(4)
            smx_b(0)
            smx_a(1)
            score_part(2)
            load_decb(5)
            smx_b(1)
            smx_a(2)
            v_group(0, 1)
            score_part(3)
            load_decb(6)
            smx_b(2)
            smx_a(3)
            v_group(1, 0)
            smx_b(3)
            load_decb(7)
            v_group(1, 1)

            # ---------------- phase 4: contexts 0-3 (ctx + dgp re-inject) --
            for n in range(NB_EARLY):
                dgp = dgp_tiles.pop(n)
                o_sb = out_pool.tile([128, 2, H], F32, tag="o", name=f"o{n}")
                dst = out[n * Q:(n + 1) * Q, :].rearrange("(qt p) h -> p qt h", p=128)
                for qt in range(2):
                    for ht in range(2):
                        po = psB.tile([128, 512], F32, tag="B", name=f"poc{n}_{qt}_{ht}")
                        ctx_mm(n, qt, ht, po, start=True)
                        nc.tensor.matmul(
                            po, lhsT=id_sb[:, :], rhs=dgp[:, qt, ht, :],
                            start=False, stop=True,
                        )
                        nc.scalar.activation(
                            out=o_sb[:, qt, ht * 512:(ht + 1) * 512],
                            in_=po, func=AF.Tanh,
                        )
                    eng = nc.sync if qt == 0 else nc.scalar
                    eng.dma_start(out=dst[:, qt, :], in_=o_sb[:, qt, :])

            # ---------------- phase 5: batches 4-7, standard pipeline ------
            prev = None  # (n, pos, o_sb) awaiting tanh + store

            def flush_prev():
                nonlocal prev
                if prev is None:
                    return
                pn, ppos, po_sb = prev
                for qt in range(2):
                    for ht in range(2):
                        nc.scalar.activation(
                            out=po_sb[:, qt, ht * 512:(ht + 1) * 512],
                            in_=ppos[(qt, ht)], func=AF.Tanh,
                        )
                dst = out[pn * Q:(pn + 1) * Q, :].rearrange("(qt p) h -> p qt h", p=128)
                nc.sync.dma_start(out=dst[:, 0, :], in_=po_sb[:, 0, :])
                nc.scalar.dma_start(out=dst[:, 1, :], in_=po_sb[:, 1, :])
                prev = None

            state = {}

            def batch_pre(n):
                score_part(n)
                flush_prev()
                o_sb = out_pool.tile([128, 2, H], F32, tag="o", name=f"o{n}")
                pos = {}

                def dec_group(qt, ht):
                    po = psB.tile([128, 512], F32, tag="B", name=f"po{n}_{qt}_{ht}")
                    pos[(qt, ht)] = po
                    dg_mms(n, qt, ht, po)

                dec_group(0, 0)
                smx_a(n)
                dec_group(1, 0)
                smx_b(n)
                dec_group(0, 1)
                dec_group(1, 1)
                state[n] = (pos, o_sb)

            def batch_ctx(n):
                pos, o_sb = state.pop(n)
                last = n == B - 1
                dst = out[n * Q:(n + 1) * Q, :].rearrange("(qt p) h -> p qt h", p=128)
                for qt in range(2):
                    for ht in range(2):
                        ctx_mm(n, qt, ht, pos[(qt, ht)], start=False)
                        if last:
                            nc.scalar.activation(
                                out=o_sb[:, qt, ht * 512:(ht + 1) * 512],
                                in_=pos[(qt, ht)], func=AF.Tanh,
                            )
                    if last:
                        eng = nc.sync if qt == 0 else nc.scalar
                        eng.dma_start(out=dst[:, qt, :], in_=o_sb[:, qt, :])
                nonlocal prev
                if not last:
                    prev = (n, pos, o_sb)

            for n in range(NB_EARLY, B):
                batch_pre(n)
                batch_ctx(n)
            flush_prev()
    nc.compile()
    return nc


def round_f32r(a: np.ndarray) -> np.ndarray:
    """Round fp32 to fp32r (TF32-like: 11-bit mantissa, low 12 bits zero),
    round-to-nearest-even.  This is what the PE consumes in fp32r mode."""
    u = np.ascontiguousarray(a, dtype=np.float32).view(np.uint32)
    lsb = (u >> np.uint32(12)) & np.uint32(1)
    u = (u + np.uint32(0x7FF) + lsb) & np.uint32(0xFFFFF000)
    return u.view(np.float32)


def prepare_in_maps(inputs: dict) -> list[dict]:
    enc = np.asarray(inputs["encoder_outputs"], dtype=np.float32)
    dec = np.asarray(inputs["decoder_h_t"], dtype=np.float32)
    src_len = np.asarray(inputs["src_len"], dtype=np.int32)
    p_t = np.asarray(inputs["p_t"], dtype=np.float32)
    W_a = np.asarray(inputs["W_a"], dtype=np.float32)
    W_c = np.asarray(inputs["W_c"], dtype=np.float32)

    # Window bounds, computed with the same fp32 ops as the reference.
    attn_start = np.maximum(p_t - np.float32(WINDOW), np.float32(0.0))
    attn_end = np.minimum(p_t + np.float32(WINDOW), src_len.astype(np.float32))
    idx_full = np.arange(L, dtype=np.float32)
    mask_full = (idx_full[None, :] < attn_start[:, None]) | (
        idx_full[None, :] > attn_end[:, None]
    )
    live = ~mask_full
    s = np.clip(live.argmax(axis=1), 0, L - W)  # first live position per batch
    idx = s[:, None] + np.arange(W)[None, :]
    idxf = idx.astype(np.float32)
    mask = (idxf < attn_start[:, None]) | (idxf > attn_end[:, None])
    bias = np.where(mask, np.float32(MASK_BIAS), np.float32(LOG_ALPHA)).astype(np.float32)
    g = np.exp(-((idxf - p_t[:, None]) ** 2) / np.float32(DEV_POW)).astype(np.float32)

    enc_w = enc[np.arange(N)[:, None], idx, :]  # [N, W, H]
    W_aTb = np.ascontiguousarray(W_a.T).astype(ml_dtypes.bfloat16)
    W_c1Tb = np.ascontiguousarray(W_c[:, :H].T).astype(ml_dtypes.bfloat16)
    W_c2Tb = np.ascontiguousarray(W_c[:, H:].T).astype(ml_dtypes.bfloat16)

    in_maps = []
    for c in range(NCORES):
        bs = slice(c * B, (c + 1) * B)
        gc = g[bs]    # [B, W]
        gpack = np.zeros((128, 2), dtype=np.float32)
        for n in range(B):
            gi, off = divmod(n, 4)
            gpack[off * W:(off + 1) * W, gi] = gc[n]
        enc_wT = np.ascontiguousarray(
            enc_w[bs].transpose(2, 0, 1).reshape(H, B * W))
        decT = np.ascontiguousarray(dec[bs].transpose(2, 0, 1).reshape(H, B * Q))
        in_maps.append({
            "enc_wTb": enc_wT.astype(ml_dtypes.bfloat16),
            "dec_bT": decT.astype(ml_dtypes.bfloat16),
            "W_aT": W_aTb,
            "W_c1T": W_c1Tb,
            "W_c2T": W_c2Tb,
            "biasT": np.ascontiguousarray(bias[bs].T),
            "onesD": np.ones((W, W), dtype=np.float32),
            "identD": np.eye(128, dtype=np.float32).astype(ml_dtypes.bfloat16),
            "gPackT": gpack,
        })
    return in_maps


_NC = None


def get_nc() -> bass.Bass:
    global _NC
    if _NC is None:
        _NC = build_nc()
    return _NC


def kernel(**inputs) -> np.ndarray:
    nc = get_nc()
    in_maps = prepare_in_maps(inputs)
    res = run_bass_kernel_spmd(nc, in_maps, list(range(NCORES)))
    outs = [res.results[c]["out"].reshape(B, Q, H) for c in range(NCORES)]
    return np.concatenate(outs, axis=0)


# revision 34
# speedup vs baseline: 1.0223x; 1.0223x over previous
"""Trainium2 Bass kernel for Luong local-p sparse attention (v5).

Math (per batch n, full shapes N=64, L=258, H=1024, Q=256):
    score = (h_t @ W_a) @ enc^T           masked to window [p_t-16, p_t+16]
    align = softmax(score) * gauss(p_t)
    out   = tanh([align @ enc, h_t] @ W_c^T)

Only a 32-wide slice of enc can survive the mask for non-integer p_t, so the
kernel gathers 32-wide windows host-side (W=32 -> 4 windows pack exactly into
128 PE partitions).  Score path (u = W_a-transform of windows, score, softmax)
runs in fp32r for softmax accuracy; the heavy W_c matmuls (dec @ W_c2T, window
@ W_c1T, align @ v) run in bf16 (same PE rate, half the DMA/SBUF).

Layout: batch n's score/softmax lives on partitions [32*(n%4), 32*(n%4)+32) so
the align weights and the W_c1-transformed windows (vst tiles, 4 windows
stacked per 128 partitions) feed the context matmul directly as quadrant tiles
— no partition-shuffling DMAs anywhere.

Schedule ("dec_group-first"): the dec @ W_c2T matmuls for batches 0-3 need
only dec + W_c2T (3 MB), so after a short PE warm-up they run PE-bound from
~14us while enc/W_aT/W_c1T stream in behind them; their partials are
evacuated to SBUF (bf16) and re-injected at context time with an identity
matmul into the same PSUM accumulation.  Then u -> scores 0-3 interleaved
with v -> contexts 0-3, and batches 4-7 run the standard softly-pipelined
score/dec_group/ctx schedule.  Data parallel over batch: 8 per core.
"""

import numpy as np
import ml_dtypes

import concourse.bass as bass
import concourse.bacc as bacc
import concourse.mybir as mybir
import concourse.tile as tile
from concourse.bass_utils import run_bass_kernel_spmd

# Problem constants (hardcoded per harness contract).
N, L, H, Q = 64, 258, 1024, 256
WINDOW = 16.0
DEV_POW = 128.0
NCORES = 8
B = N // NCORES  # batches per core
W = 32           # window width (max live positions for non-integer p_t)
HC = H // 128    # h-chunks of 128 (PE contraction tiles)
F32 = mybir.dt.float32
F32R = mybir.dt.float32r
BF16 = mybir.dt.bfloat16
F16 = mybir.dt.float16
AF = mybir.ActivationFunctionType

# exp is computed as t = exp(s/4 + bias); bias = LOG_ALPHA keeps the
# column-sum T = sum_j t below fp32 max.  alpha cancels in w = t/T.
LOG_ALPHA = -4.8520302  # -7*ln(2)
MASK_BIAS = -10000.0    # exp(<= -2500) == 0 in fp32
N_WARM = 34             # PE warm-up matmuls spanning the initial DMA wait
NB_EARLY = 4            # batches whose dec_group runs ahead of u/score/v


def build_nc() -> bass.Bass:
    nc = bacc.Bacc()
    dec_bT = nc.declare_dram_parameter("dec_bT", [H, B * Q], BF16, isOutput=False)
    enc_wTb = nc.declare_dram_parameter("enc_wTb", [H, B * W], BF16, isOutput=False)
    W_aT = nc.declare_dram_parameter("W_aT", [H, H], BF16, isOutput=False)
    W_c1T = nc.declare_dram_parameter("W_c1T", [H, H], BF16, isOutput=False)
    W_c2T = nc.declare_dram_parameter("W_c2T", [H, H], BF16, isOutput=False)
    biasT = nc.declare_dram_parameter("biasT", [W, B], F32, isOutput=False)
    gPackT = nc.declare_dram_parameter("gPackT", [128, 2], F32, isOutput=False)
    onesD = nc.declare_dram_parameter("onesD", [W, W], F32R, isOutput=False)
    identD = nc.declare_dram_parameter("identD", [128, 128], BF16, isOutput=False)
    out = nc.declare_dram_parameter("out", [B * Q, H], F32, isOutput=True)

    with tile.TileContext(nc) as tc:
        with (
            tc.tile_pool(name="const", bufs=1) as cpool,
            tc.tile_pool(name="decb", bufs=4) as decb_pool,
            tc.tile_pool(name="dgp", bufs=NB_EARLY) as dgp_pool,
            tc.tile_pool(name="sm", bufs=2) as sm_pool,
            tc.tile_pool(name="outp", bufs=2) as out_pool,
            tc.tile_pool(name="psS", bufs=2, space="PSUM") as psS,
            tc.tile_pool(name="psB", bufs=6, space="PSUM") as psB,
        ):
            # ---------------- resident tensors ----------------
            encb_sb = cpool.tile([128, HC, B * W], BF16)
            WaT_sb = cpool.tile([128, HC, H], BF16)
            Wc1_sb = cpool.tile([128, HC, H], BF16)
            Wc2_sb = cpool.tile([128, HC, H], BF16)
            uT_sb = cpool.tile([128, HC, B * W], BF16)
            vst_sb = cpool.tile([128, 2, H], BF16)   # [4 windows, group, h']
            tb_sb = cpool.tile([128, 2, Q], BF16)    # align weights, stacked
            bias_sb = cpool.tile([W, B], F32)
            gpack_sb = cpool.tile([128, 2], F32)
            ones_sb = cpool.tile([W, W], F32R)
            id_sb = cpool.tile([128, 128], BF16)
            warm_sb = cpool.tile([128, 640], BF16)

            WaT_r = W_aT[:, :].rearrange("(c p) m -> p c m", p=128)
            Wc1_r = W_c1T[:, :].rearrange("(c p) m -> p c m", p=128)
            Wc2_r = W_c2T[:, :].rearrange("(c p) m -> p c m", p=128)
            decb_r = dec_bT[:, :].rearrange("(c p) (n q) -> p c n q", p=128, q=Q)
            encb_r = enc_wTb[:, :].rearrange("(c p) m -> p c m", p=128)

            decb_tiles = {}

            def load_decb(n):
                dt_ = decb_pool.tile([128, HC, Q], BF16, tag="decb", name=f"decb{n}")
                nc.sync.dma_start(out=dt_[:, 0:5, :], in_=decb_r[:, 0:5, n, :])
                nc.scalar.dma_start(out=dt_[:, 5:8, :], in_=decb_r[:, 5:8, n, :])
                decb_tiles[n] = dt_

            # ---------------- DMA schedule (issue order = priority) --------
            # tiny constants first (scalar ring)
            nc.scalar.dma_start(out=bias_sb, in_=biasT[:, :])
            nc.scalar.dma_start(out=gpack_sb, in_=gPackT[:, :])
            nc.scalar.dma_start(out=ones_sb, in_=onesD[:, :])
            nc.scalar.dma_start(out=id_sb, in_=identD[:, :])

            # dec_group-first deps: decb0, W_c2T, decb1-3 (all bf16)
            load_decb(0)
            nc.sync.dma_start(out=Wc2_sb[:, 0:3, :], in_=Wc2_r[:, 0:3, :])
            nc.scalar.dma_start(out=Wc2_sb[:, 3:5, :], in_=Wc2_r[:, 3:5, :])
            nc.sync.dma_start(out=Wc2_sb[:, 5:8, :], in_=Wc2_r[:, 5:8, :])
            for n in range(1, NB_EARLY):
                load_decb(n)

            # u deps: bf16 windows + W_aT (kc-major, both queues)
            nc.sync.dma_start(out=encb_sb[:, 0:5, :], in_=encb_r[:, 0:5, :])
            nc.scalar.dma_start(out=encb_sb[:, 5:8, :], in_=encb_r[:, 5:8, :])
            nc.sync.dma_start(out=WaT_sb[:, 0:3, :], in_=WaT_r[:, 0:3, :])
            nc.scalar.dma_start(out=WaT_sb[:, 3:5, :], in_=WaT_r[:, 3:5, :])
            nc.sync.dma_start(out=WaT_sb[:, 5:8, :], in_=WaT_r[:, 5:8, :])

            # v deps: W_c1T
            nc.scalar.dma_start(out=Wc1_sb[:, 0:4, :], in_=Wc1_r[:, 0:4, :])
            nc.sync.dma_start(out=Wc1_sb[:, 4:8, :], in_=Wc1_r[:, 4:8, :])

            # ---------------- PE warm-up ----------------
            # Long back-to-back matmuls cycling all 6 big PSUM slots (deep
            # pipelining hides slot-reuse semaphores); sustained PE busy trips
            # the HAM clock gate to 8/8 before real work starts.
            nc.vector.memset(warm_sb[:, :], 1.0)
            for i in range(N_WARM):
                pw = psB.tile([128, 512], F32, tag="B", name=f"warm{i}")
                nc.tensor.matmul(
                    pw, lhsT=warm_sb[:, 0:128], rhs=warm_sb[:, 128:640],
                    start=True, stop=True,
                )

            # ---------------- helpers ----------------
            def dg_mms(n, qt, ht, po, stop_last=False):
                db = decb_tiles[n]
                for hc in range(HC):
                    nc.tensor.matmul(
                        po,
                        lhsT=db[:, hc, qt * 128:(qt + 1) * 128],
                        rhs=Wc2_sb[:, hc, ht * 512:(ht + 1) * 512],
                        start=(hc == 0),
                        stop=(stop_last and hc == HC - 1),
                    )

            t_tiles = {}

            def score_part(n):
                dec_sb = decb_tiles[n]
                ps = psS.tile([W, Q], F32, tag="S", name=f"ps{n}")
                for hc in range(HC):
                    nc.tensor.matmul(
                        ps,
                        lhsT=uT_sb[:, hc, n * W:(n + 1) * W],
                        rhs=dec_sb[:, hc, :],
                        start=(hc == 0),
                        stop=(hc == HC - 1),
                    )
                t = sm_pool.tile([W, Q], F32R, tag="t", name=f"t{n}")
                nc.scalar.activation(
                    out=t, in_=ps, func=AF.Exp,
                    bias=bias_sb[:, n:n + 1], scale=0.25,
                )
                t_tiles[n] = t

            def smx_a(n):
                # first renorm pass: T = colsum t; t = (t/T)^4 (two squarings)
                t = t_tiles[n]
                pT = psS.tile([W, Q], F32, tag="S", name=f"pT{n}")
                nc.tensor.matmul(pT, lhsT=ones_sb, rhs=t, start=True, stop=True)
                rT = sm_pool.tile([W, Q], F32, tag="r", name=f"rT{n}")
                nc.vector.reciprocal_approx_fast(out=rT, in_=pT)
                nc.vector.tensor_mul(t, t, rT)
                nc.vector.tensor_mul(t, t, t)
                nc.vector.tensor_mul(t, t, t)

            def smx_b(n):
                # second renorm pass -> bf16 align weights; a tiny SWDGE DMA
                # shifts them into the stacked [4 windows x 128] ctx layout
                o = (n % 4) * W
                g = n // 4
                t = t_tiles[n]
                pZ = psS.tile([W, Q], F32, tag="S", name=f"pZ{n}")
                nc.tensor.matmul(pZ, lhsT=ones_sb, rhs=t, start=True, stop=True)
                rZ = sm_pool.tile([W, Q], F32, tag="r", name=f"rZ{n}")
                nc.vector.reciprocal_approx_fast(out=rZ, in_=pZ)
                tbt = sm_pool.tile([W, Q], BF16, tag="tbt", name=f"tbt{n}")
                nc.vector.tensor_mul(tbt, t, rZ)
                nc.gpsimd.dma_start(out=tb_sb[o:o + W, g, :], in_=tbt)
                del t_tiles[n]

            def v_group(g, nt):
                pv = psB.tile([128, 512], F32, tag="B", name=f"pv{g}_{nt}")
                for kc in range(HC):
                    nc.tensor.matmul(
                        pv,
                        lhsT=encb_sb[:, kc, g * 128:(g + 1) * 128],
                        rhs=Wc1_sb[:, kc, nt * 512:(nt + 1) * 512],
                        start=(kc == 0),
                        stop=(kc == HC - 1),
                    )
                # fold the gaussian; vst stays resident as the ctx rhs
                nc.vector.tensor_scalar_mul(
                    vst_sb[:, g, nt * 512:(nt + 1) * 512], pv, gpack_sb[:, g:g + 1]
                )

            def ctx_mm(n, qt, ht, po, start):
                o = (n % 4) * W
                g = n // 4
                nc.tensor.matmul(
                    po,
                    lhsT=tb_sb[o:o + W, g, qt * 128:(qt + 1) * 128],
                    rhs=vst_sb[o:o + W, g, ht * 512:(ht + 1) * 512],
                    start=start,
                    stop=not start,
                    tile_position=(o, 0),
                )

            # ---------------- phase 1: dec_group for batches 0-3 ----------
            # Partials evacuated to SBUF bf16; re-injected at context time.
            dgp_tiles = {}
            for n in range(NB_EARLY):
                dgp = dgp_pool.tile([128, 2, 2, 512], BF16, tag="dgp", name=f"dgp{n}")
                dgp_tiles[n] = dgp
                for qt, ht in ((0, 0), (1, 0), (0, 1), (1, 1)):
                    po = psB.tile([128, 512], F32, tag="B", name=f"pod{n}_{qt}_{ht}")
                    dg_mms(n, qt, ht, po, stop_last=True)
                    nc.scalar.copy(out=dgp[:, qt, ht, :], in_=po)

            # ---------------- phase 2: u  (uT[h, (n,j)], kc-major waves) ---
            for wave in range(2):
                pu = {}
                for kc in range(HC):
                    for ho in range(4):
                        hco = wave * 4 + ho
                        if kc == 0:
                            pu[hco] = psB.tile(
                                [128, B * W], F32, tag="B", name=f"pu{hco}"
                            )
                        nc.tensor.matmul(
                            pu[hco],
                            lhsT=WaT_sb[:, kc, hco * 128:(hco + 1) * 128],
                            rhs=encb_sb[:, kc, :],
                            start=(kc == 0),
                            stop=(kc == HC - 1),
                        )
                for ho in range(4):
                    hco = wave * 4 + ho
                    if ho % 2 == 0:
                        nc.scalar.copy(out=uT_sb[:, hco, :], in_=pu[hco])
                    else:
                        nc.vector.tensor_copy(out=uT_sb[:, hco, :], in_=pu[hco])

            # ------- phase 3: scores+softmax 0-3 interleaved with v ----
            v_group(0, 0)
            score_part(0)
            smx_a(0)
            score_part(1)
            load_decb(4)
            smx_b(0)
            smx_a(1)
            score_part(2)
            load_decb(5)
            smx_b(1)
            smx_a(2)
            v_group(0, 1)
            score_part(3)
            load_decb(6)
            smx_b(2)
            smx_a(3)
            v_group(1, 0)
            smx_b(3)
            load_decb(7)
            v_group(1, 1)

            # ---------------- phase 4: contexts 0-3 (ctx + dgp re-inject) --
            for n in range(NB_EARLY):
                dgp = dgp_tiles.pop(n)
                o_sb = out_pool.tile([128, 2, H], F32, tag="o", name=f"o{n}")
                dst = out[n * Q:(n + 1) * Q, :].rearrange("(qt p) h -> p qt h", p=128)
                for qt in range(2):
                    for ht in range(2):
                        po = psB.tile([128, 512], F32, tag="B", name=f"poc{n}_{qt}_{ht}")
                        ctx_mm(n, qt, ht, po, start=True)
                        nc.tensor.matmul(
                            po, lhsT=id_sb[:, :], rhs=dgp[:, qt, ht, :],
                            start=False, stop=True,
                        )
                        nc.scalar.activation(
                            out=o_sb[:, qt, ht * 512:(ht + 1) * 512],
                            in_=po, func=AF.Tanh,
                        )
                    eng = nc.sync if qt == 0 else nc.scalar
                    eng.dma_start(out=dst[:, qt, :], in_=o_sb[:, qt, :])

            # ---------------- phase 5: batches 4-7, standard pipeline ------
            prev = None  # (n, pos, o_sb) awaiting tanh + store

            def flush_prev():
                nonlocal prev
                if prev is None:
                    return
                pn, ppos, po_sb = prev
                for qt in range(2):
                    for ht in range(2):
                        nc.scalar.activation(
                            out=po_sb[:, qt, ht * 512:(ht + 1) * 512],
                            in_=ppos[(qt, ht)], func=AF.Tanh,
                        )
                dst = out[pn * Q:(pn + 1) * Q, :].rearrange("(qt p) h -> p qt h", p=128)
                nc.sync.dma_start(out=dst[:, 0, :], in_=po_sb[:, 0, :])
                nc.scalar.dma_start(out=dst[:, 1, :], in_=po_sb[:, 1, :])
                prev = None

            state = {}

            def batch_pre(n):
                score_part(n)
                flush_prev()
                o_sb = out_pool.tile([128, 2, H], F32, tag="o", name=f"o{n}")
                pos = {}

                def dec_group(qt, ht):
                    po = psB.tile([128, 512], F32, tag="B", name=f"po{n}_{qt}_{ht}")
                    pos[(qt, ht)] = po
                    dg_mms(n, qt, ht, po)

                dec_group(0, 0)
                smx_a(n)
                dec_group(1, 0)
                smx_b(n)
                dec_group(0, 1)
                dec_group(1, 1)
                state[n] = (pos, o_sb)

            def batch_ctx(n):
                pos, o_sb = state.pop(n)
                last = n == B - 1
                dst = out[n * Q:(n + 1) * Q, :].rearrange("(qt p) h -> p qt h", p=128)
                for qt in range(2):
                    for ht in range(2):
                        ctx_mm(n, qt, ht, pos[(qt, ht)], start=False)
                        if last:
                            nc.scalar.activation(
                                out=o_sb[:, qt, ht * 512:(ht + 1) * 512],
                                in_=pos[(qt, ht)], func=AF.Tanh,
                            )
                    if last:
                        eng = nc.sync if qt == 0 else nc.scalar
                        eng.dma_start(out=dst[:, qt, :], in_=o_sb[:, qt, :])
                nonlocal prev
                if not last:
                    prev = (n, pos, o_sb)

            for n in range(NB_EARLY, B):
                batch_pre(n)
                batch_ctx(n)
            flush_prev()
    nc.compile()
    return nc


def round_f32r(a: np.ndarray) -> np.ndarray:
    """Round fp32 to fp32r (TF32-like: 11-bit mantissa, low 12 bits zero),
    round-to-nearest-even.  This is what the PE consumes in fp32r mode."""
    u = np.ascontiguousarray(a, dtype=np.float32).view(np.uint32)
    lsb = (u >> np.uint32(12)) & np.uint32(1)
    u = (u + np.uint32(0x7FF) + lsb) & np.uint32(0xFFFFF000)
    return u.view(np.float32)


def prepare_in_maps(inputs: dict) -> list[dict]:
    enc = np.asarray(inputs["encoder_outputs"], dtype=np.float32)
    dec = np.asarray(inputs["decoder_h_t"], dtype=np.float32)
    src_len = np.asarray(inputs["src_len"], dtype=np.int32)
    p_t = np.asarray(inputs["p_t"], dtype=np.float32)
    W_a = np.asarray(inputs["W_a"], dtype=np.float32)
    W_c = np.asarray(inputs["W_c"], dtype=np.float32)

    # Window bounds, computed with the same fp32 ops as the reference.
    attn_start = np.maximum(p_t - np.float32(WINDOW), np.float32(0.0))
    attn_end = np.minimum(p_t + np.float32(WINDOW), src_len.astype(np.float32))
    idx_full = np.arange(L, dtype=np.float32)
    mask_full = (idx_full[None, :] < attn_start[:, None]) | (
        idx_full[None, :] > attn_end[:, None]
    )
    live = ~mask_full
    s = np.clip(live.argmax(axis=1), 0, L - W)  # first live position per batch
    idx = s[:, None] + np.arange(W)[None, :]
    idxf = idx.astype(np.float32)
    mask = (idxf < attn_start[:, None]) | (idxf > attn_end[:, None])
    bias = np.where(mask, np.float32(MASK_BIAS), np.float32(LOG_ALPHA)).astype(np.float32)
    g = np.exp(-((idxf - p_t[:, None]) ** 2) / np.float32(DEV_POW)).astype(np.float32)

    enc_w = enc[np.arange(N)[:, None], idx, :]  # [N, W, H]
    W_aTb = np.ascontiguousarray(W_a.T).astype(ml_dtypes.bfloat16)
    W_c1Tb = np.ascontiguousarray(W_c[:, :H].T).astype(ml_dtypes.bfloat16)
    W_c2Tb = np.ascontiguousarray(W_c[:, H:].T).astype(ml_dtypes.bfloat16)

    in_maps = []
    for c in range(NCORES):
        bs = slice(c * B, (c + 1) * B)
        gc = g[bs]    # [B, W]
        gpack = np.zeros((128, 2), dtype=np.float32)
        for n in range(B):
            gi, off = divmod(n, 4)
            gpack[off * W:(off + 1) * W, gi] = gc[n]
        enc_wT = np.ascontiguousarray(
            enc_w[bs].transpose(2, 0, 1).reshape(H, B * W))
        decT = np.ascontiguousarray(dec[bs].transpose(2, 0, 1).reshape(H, B * Q))
        in_maps.append({
            "enc_wTb": enc_wT.astype(ml_dtypes.bfloat16),
            "dec_bT": decT.astype(ml_dtypes.bfloat16),
            "W_aT": W_aTb,
            "W_c1T": W_c1Tb,
            "W_c2T": W_c2Tb,
            "biasT": np.ascontiguousarray(bias[bs].T),
            "onesD": np.ones((W, W), dtype=np.float32),
            "identD": np.eye(128, dtype=np.float32).astype(ml_dtypes.bfloat16),
            "gPackT": gpack,
        })
    return in_maps


_NC = None


def get_nc() -> bass.Bass:
    global _NC
    if _NC is None:
        _NC = build_nc()
    return _NC


def kernel(**inputs) -> np.ndarray:
    nc = get_nc()
    in_maps = prepare_in_maps(inputs)
    res = run_bass_kernel_spmd(nc, in_maps, list(range(NCORES)))
    outs = [res.results[c]["out"].reshape(B, Q, H) for c in range(NCORES)]
    return np.concatenate(outs, axis=0)
